# revision 13
# baseline (speedup 1.0000x reference)
"""Bass/Trainium2 kernel for nn_Encoders_6751688590031 (v2).

4-layer transformer encoder, d_model=64, H=8 heads, L=1024, dff=256, B=8.
Sharding: data-parallel over batch across 8 NeuronCores (1 batch element per
core); weights replicated. One tiny AllReduce(max) per layer for the global
jnp.max(w) softmax normalization (plus a warmup AllReduce to absorb the
first-use collective setup cost).

Design notes (vs reference.py math):
 - two_d_softm max-subtraction skipped (logits stay within +-40): per-(b,h)
   stats needed are s_h = sum(e), mx_h = max(e); global scale
   G = max_bh nz*mx_h/s_h (AllReduce max); out = (e^T v) * nz/(s_h*G).
 - all biases and the keep/padneg masks are folded into augmented matmuls:
   xT has rows [x^T (masked); keepf; ones], W*_aug carry bias rows and the
   +-1e9 rows that reproduce the -1e9 padding additive mask exactly.
 - e^T v is computed with v as the stationary operand ([128,32] tiles:
   8 v-cols + ones col + zero pad) and e as the 512-col bf16 moving operand,
   with each head's output placed at PSUM partition strip 32*(h%4) of one of
   two PSUM tiles (heads 0-3 / 4-7) via matmul tile_position. The result IS
   attn^T in feature-major layout - no transposes, and the ones column gives
   the softmax sums for free.
 - the nz/(s*G) scale is applied to the Wo weights (rebuilt per layer from a
   fp32 master via one tensor_scalar), so the AllReduce is off the critical
   path of the attnT evacuation.
 - everything the PE streams is bf16 (f32r measured ~2x slower per column).
"""

import os
import sys

import numpy as np

for _p in (
    "/root/.axon_site",
    "/root/.axon_site/_ro/trn_rl_repo",
    "/root/.axon_site/_ro/pypackages",
    "/opt/trn_rl_repo",
):
    if os.path.isdir(_p) and _p not in sys.path:
        sys.path.append(_p)

import ml_dtypes
import concourse.bass as bass
import concourse.bacc as bacc
import concourse.tile as tile
from concourse import mybir

F32 = mybir.dt.float32
BF16 = mybir.dt.bfloat16
BF = ml_dtypes.bfloat16

L = 1024          # sequence length
D = 64            # d_model
H = 8             # heads
DH = 8            # head dim
DFF = 256
NL = 4            # layers
P = 128           # partitions per token tile
NT = L // P       # 8 token tiles
AUG = 128         # q/k row pitch: head (t,m) at rows 32m..32m+7, ones/padneg at 32m+8
NQT = 2           # q/k head-group tiles (heads 0-3, 4-7)
XAUG = 66         # xT rows: 64 features + keepf row + ones row
HPT = (4, 4)      # heads per q/k tile
NCORES = 8
NEG_BIG = 1.0e9
LN_EPS = 1e-9
GPS_HEADS = (2, 4, 5, 6)   # heads whose max-reduce is folded on gpsimd

Act = mybir.ActivationFunctionType
Alu = mybir.AluOpType



def ap_raw(t, ap):
    return bass.AP(tensor=t.tensor, offset=t.offset, ap=ap)


def bc_j(t, reps):
    """[P, f] tile -> [P, reps, f] view with stride-0 middle dim."""
    return ap_raw(t, [t.ap[0], [0, reps]] + t.ap[1:])


def bc_f(t, reps):
    """[P, n] tile -> [P, n, reps] view with stride-0 inner dim."""
    return ap_raw(t, t.ap[:-1] + [t.ap[-1], [0, reps]])


def build_bass():
    nc = bacc.Bacc(
        "TRN2", target_bir_lowering=False, debug=False, num_devices=NCORES
    )

    # ---- parameters (per core) ----
    x_in = nc.declare_dram_parameter("x", [L, D], F32, isOutput=False)
    keepf = nc.declare_dram_parameter("keepf", [L], F32, isOutput=False)
    keep_bf = nc.declare_dram_parameter("keep_bf", [L], BF16, isOutput=False)
    Wq = nc.declare_dram_parameter("Wq", [NL, NQT, XAUG, AUG], BF16, isOutput=False)
    Wk = nc.declare_dram_parameter("Wk", [NL, NQT, XAUG, AUG], BF16, isOutput=False)
    Wv = nc.declare_dram_parameter("Wv", [NL, XAUG, D], BF16, isOutput=False)
    WoM = nc.declare_dram_parameter("WoM", [NL, 2, P, D], F32, isOutput=False)
    bov = nc.declare_dram_parameter("bov", [NL, D], F32, isOutput=False)
    W1 = nc.declare_dram_parameter("W1", [NL, XAUG - 1, DFF], BF16, isOutput=False)
    W2 = nc.declare_dram_parameter("W2", [NL, 2, P, D], BF16, isOutput=False)
    b2_r = nc.declare_dram_parameter("b2_r", [NL, 1, D], BF16, isOutput=False)
    g1v = nc.declare_dram_parameter("g1v", [NL, D], F32, isOutput=False)
    be1v = nc.declare_dram_parameter("be1v", [NL, D], F32, isOutput=False)
    g2v = nc.declare_dram_parameter("g2v", [NL, D], F32, isOutput=False)
    be2v = nc.declare_dram_parameter("be2v", [NL, D], F32, isOutput=False)
    e128 = nc.declare_dram_parameter("e128", [2, DH, P], F32, isOutput=False)
    sel128 = nc.declare_dram_parameter("sel128", [2, P, DH], F32, isOutput=False)
    out = nc.declare_dram_parameter("out", [L, D], F32, isOutput=True)

    dma = nc.sync.dma_start

    with tile.TileContext(nc) as tc:
        with (
            tc.tile_pool(name="const", bufs=1) as constp,
            tc.tile_pool(name="wpool", bufs=1) as wp,
            tc.tile_pool(name="pers", bufs=1) as pers,
            tc.tile_pool(name="acts", bufs=2) as acts,
            tc.tile_pool(name="epool", bufs=2) as epool,
            tc.tile_pool(name="fold", bufs=3) as foldp,
            tc.tile_pool(name="stats", bufs=2) as stats,
            tc.tile_pool(name="lnw", bufs=2) as lnwp,
            tc.tile_pool(name="psL", bufs=2, space="PSUM") as psL,
            tc.tile_pool(name="psA", bufs=1, space="PSUM") as psA,
            tc.tile_pool(name="dram", bufs=1, space="DRAM") as dramp,
        ):
            # ---- initial x load: token-major packed [128, (j,f)]; FIRST ----
            x_all = acts.tile([P, NT * D], F32, name="x_all0", tag="x")
            dma(out=x_all.rearrange("p (j f) -> p j f", f=D),
                in_=x_in.rearrange("(j p) f -> p j f", p=P))
            xT = pers.tile([XAUG, L], BF16, name="xT")
            dma(out=xT[D: D + 1, :], in_=keep_bf.rearrange("(o n) -> o n", o=1))

            # ================= constants =================
            ones_t = constp.tile([P, P], F32, name="ones_t")
            nc.vector.memset(ones_t, 1.0)
            I128 = constp.tile([P, P], F32, name="I128")
            nc.gpsimd.affine_select(
                out=I128, in_=ones_t, pattern=[[-1, P]],
                compare_op=Alu.is_equal, fill=0.0, base=0, channel_multiplier=1,
            )
            ones_row8 = constp.tile([1, H], F32, name="ones_row8")
            nc.vector.memset(ones_row8, 1.0)
            ones_bf = constp.tile([P, L], BF16, name="ones_bf")
            nc.vector.memset(ones_bf, 1.0)

            # ===== warmup collective (absorbs CC setup cost; no readback) ===
            wgl = constp.tile([1, 1], F32, name="wgl")
            nc.vector.memset(wgl, 0.0)
            ccw_in = dramp.tile([1, 1], F32, name="ccw_in", tag="ccw_in")
            ccw_out = dramp.tile([1, 1], F32, name="ccw_out", tag="ccw_out",
                                 addr_space="Shared")
            nc.gpsimd.dma_start(out=ccw_in[:], in_=wgl)
            nc.gpsimd.collective_compute(
                "AllReduce", Alu.max,
                replica_groups=[list(range(NCORES))],
                ins=[ccw_in.opt()], outs=[ccw_out.opt()],
            )

            # token-major keep: keep_all[p, j] = keepf[j*128+p]
            keep_JP = constp.tile([NT, P], F32, name="keep_JP")
            dma(out=keep_JP, in_=keepf.rearrange("(j p) -> j p", p=P))
            pKA = psL.tile([P, L], F32, name="pKA", tag="L")
            nc.tensor.transpose(out=pKA[:, 0:NT], in_=keep_JP,
                                identity=I128[0:NT, 0:NT])
            keep_all = constp.tile([P, NT], F32, name="keep_all")
            nc.vector.tensor_copy(keep_all, pKA[:, 0:NT])
            # keep_exp[p, j, f] = keepf[j*128+p]
            keep_exp = constp.tile([P, NT, D], F32, name="keep_exp")
            for j in range(NT):
                nc.vector.tensor_scalar(
                    out=keep_exp[:, j, :], in0=ones_t[:, 0:D],
                    scalar1=keep_all[:, j: j + 1], scalar2=None, op0=Alu.mult,
                )
            keep_exp2 = keep_exp.rearrange("p j f -> p (j f)")

            # nz broadcast to [8,1]
            nzk = constp.tile([P, 1], F32, name="nzk")
            nc.vector.reduce_sum(out=nzk, in_=keep_all, axis=mybir.AxisListType.X)
            pNZ = psL.tile([P, L], F32, name="pNZ", tag="L")
            nc.tensor.matmul(pNZ[0:1, 0:1], ones_t[:, 0:1], nzk)
            nz1 = constp.tile([1, 1], F32, name="nz1")
            nc.vector.tensor_copy(nz1, pNZ[0:1, 0:1])
            pNZ8 = psL.tile([P, L], F32, name="pNZ8", tag="L")
            nc.tensor.matmul(pNZ8[0:H, 0:1], ones_row8, nz1)
            nz8 = constp.tile([H, 1], F32, name="nz8")
            nc.vector.tensor_copy(nz8, pNZ8[0:H, 0:1])

            # selection/expansion consts
            e128X = constp.tile([DH, P], F32, name="e128X")
            dma(out=e128X, in_=e128[0])
            e128Y = constp.tile([DH, P], F32, name="e128Y")
            dma(out=e128Y, in_=e128[1])
            selX = constp.tile([P, DH], F32, name="selX")
            dma(out=selX, in_=sel128[0])
            selY = constp.tile([P, DH], F32, name="selY")
            dma(out=selY, in_=sel128[1])

            # ================= weights =================
            def bcast64(dram_vec, name, eng=nc.scalar):
                t = wp.tile([P, D], F32, name=name)
                eng.dma_start(out=t, in_=ap_raw(dram_vec, [[0, P]] + dram_vec.ap))
                return t

            wq_t, wk_t, wv_t, woX_t, woY_t = [], [], [], [], []
            bo_t, w1_t, w2_t, b2_t = [], [], [], []
            g1_t, be1_t, g2_t, be2_t = [], [], [], []
            for l in range(NL):
                wq = []
                wk = []
                for t in range(NQT):
                    wqx = wp.tile([XAUG, AUG], BF16, name=f"wq{l}_{t}")
                    dma(out=wqx, in_=Wq[l, t])
                    wq.append(wqx)
                    wkx = wp.tile([XAUG, AUG], BF16, name=f"wk{l}_{t}")
                    dma(out=wkx, in_=Wk[l, t])
                    wk.append(wkx)
                wv = wp.tile([XAUG, D], BF16, name=f"wv{l}")
                dma(out=wv, in_=Wv[l])
                woX = wp.tile([P, D], F32, name=f"woX{l}")
                nc.gpsimd.dma_start(out=woX, in_=WoM[l, 0])
                woY = wp.tile([P, D], F32, name=f"woY{l}")
                nc.gpsimd.dma_start(out=woY, in_=WoM[l, 1])
                bo = bcast64(bov[l], f"bo{l}")
                w1 = wp.tile([XAUG - 1, DFF], BF16, name=f"w1{l}")
                dma(out=w1, in_=W1[l])
                w2a = wp.tile([P, D], BF16, name=f"w2a{l}")
                dma(out=w2a, in_=W2[l, 0])
                w2b = wp.tile([P, D], BF16, name=f"w2b{l}")
                dma(out=w2b, in_=W2[l, 1])
                b2 = wp.tile([1, D], BF16, name=f"b2{l}")
                nc.gpsimd.dma_start(out=b2, in_=b2_r[l])
                g1b = bcast64(g1v[l], f"g1b{l}")
                be1b = bcast64(be1v[l], f"be1b{l}")
                g2b = bcast64(g2v[l], f"g2b{l}")
                be2b = bcast64(be2v[l], f"be2b{l}")
                wq_t.append(wq); wk_t.append(wk); wv_t.append(wv)
                woX_t.append(woX); woY_t.append(woY); bo_t.append(bo)
                w1_t.append(w1); w2_t.append((w2a, w2b)); b2_t.append(b2)
                g1_t.append(g1b); be1_t.append(be1b)
                g2_t.append(g2b); be2_t.append(be2b)

            # ================= persistent activation tiles =================
            nc.gpsimd.dma_start(out=xT[D + 1: D + 2, :], in_=ones_bf[0:1, :])
            out1T = pers.tile([XAUG - 1, L], BF16, name="out1T")
            nc.vector.tensor_copy(out1T[D: D + 1, :], ones_bf[0:1, :])
            # vt32[p, j, h, c]: c 0-7 = v_h token-major, c 8 = 1.0, c 9-31 = 0
            vt32 = pers.tile([P, NT, H, 32], BF16, name="vt32")
            nc.vector.memset(vt32, 0.0)
            nc.vector.memset(vt32[:, :, :, 8:9], 1.0)
            attnTX = pers.tile([P, L], BF16, name="attnTX")
            attnTY = pers.tile([P, L], BF16, name="attnTY")
            qaT = [pers.tile([AUG, L], BF16, name=f"qaT{t}") for t in range(NQT)]
            kaT = [pers.tile([AUG, L], BF16, name=f"kaT{t}") for t in range(NQT)]

            for l in range(NL):
                # ======== xT rows 0-63 (feature-major, masked) ========
                if l == 0:
                    xsrc = acts.tile([P, NT * D], F32, name="xm0", tag="xm")
                    nc.vector.tensor_mul(xsrc, x_all, keep_exp2)
                else:
                    xsrc = x_all  # LN2 output is already keep-masked
                pXT = psL.tile([P, L], F32, name=f"pXT{l}", tag="L")
                for j in range(NT):
                    nc.tensor.transpose(
                        out=pXT[0:D, j * P:(j + 1) * P],
                        in_=xsrc[:, j * D:(j + 1) * D], identity=I128,
                    )
                for hf in range(2):
                    csl = slice(hf * 512, (hf + 1) * 512)
                    nc.vector.tensor_copy(xT[0:D, csl], pXT[0:D, csl])

                # ======== QKV projections ========
                for t in range(NQT):
                    pQ = psL.tile([P, L], F32, name=f"pQ{l}_{t}", tag="L")
                    for hf in range(2):
                        nc.tensor.matmul(pQ[:, hf * 512:(hf + 1) * 512],
                                         wq_t[l][t],
                                         xT[:, hf * 512:(hf + 1) * 512])
                    for hf in range(2):
                        csl = slice(hf * 512, (hf + 1) * 512)
                        (nc.vector.tensor_copy if t == 0 else nc.scalar.copy)(
                            qaT[t][:, csl], pQ[:, csl])
                    pK = psL.tile([P, L], F32, name=f"pK{l}_{t}", tag="L")
                    for hf in range(2):
                        nc.tensor.matmul(pK[:, hf * 512:(hf + 1) * 512],
                                         wk_t[l][t],
                                         xT[:, hf * 512:(hf + 1) * 512])
                    for hf in range(2):
                        csl = slice(hf * 512, (hf + 1) * 512)
                        (nc.vector.tensor_copy if t == 0 else nc.scalar.copy)(
                            kaT[t][:, csl], pK[:, csl])

                # v direct to token-major: out[q, d] = sum_f xT[f, q] Wv[f, d]
                pV = psL.tile([P, L], F32, name=f"pV{l}", tag="L")
                for j in range(NT):
                    nc.tensor.matmul(pV[:, j * D:(j + 1) * D],
                                     xT[:, j * P:(j + 1) * P], wv_t[l])
                nc.vector.tensor_copy(
                    vt32[:, :, :, 0:8],
                    pV[:, 0:NT * D].rearrange("p (j h c) -> p j h c", h=H, c=DH))

                # x + bo (residual with bias pre-added; 1/G scale excludes bo)
                x_bo = acts.tile([P, NT * D], F32, name=f"xbo{l}", tag="xbo")
                nc.vector.tensor_tensor(
                    out=x_bo.rearrange("p (j f) -> p j f", f=D),
                    in0=x_all.rearrange("p (j f) -> p j f", f=D),
                    in1=bc_j(bo_t[l], NT), op=Alu.add)

                # LN2 keep-folded gamma/beta for this layer
                g2k = lnwp.tile([P, NT * D], F32, name=f"g2k{l}", tag="g2k")
                nc.gpsimd.tensor_tensor(
                    out=g2k.rearrange("p (j f) -> p j f", f=D), in0=keep_exp,
                    in1=bc_j(g2_t[l], NT), op=Alu.mult)
                be2k = lnwp.tile([P, NT * D], F32, name=f"be2k{l}", tag="be2k")
                nc.gpsimd.tensor_tensor(
                    out=be2k.rearrange("p (j f) -> p j f", f=D), in0=keep_exp,
                    in1=bc_j(be2_t[l], NT), op=Alu.mult)

                # ======== attention head loop ========
                mx_s8 = stats.tile([P, H], F32, name=f"mxs{l}", tag="mxs")
                e_tiles = {}
                run_tiles = {}
                X = psA.tile([P, L], F32, name=f"X{l}", tag="X")
                Y = psA.tile([P, L], F32, name=f"Y{l}", tag="Y")

                def logits_head(h):
                    t, m = divmod(h, 4)
                    rb = 32 * m
                    e_h = epool.tile([P, NT * L], BF16, name=f"e{l}_{h}", tag="e")
                    e_tiles[h] = e_h
                    run = foldp.tile([P, L], BF16, name=f"run{l}_{h}", tag="run")
                    run_tiles[h] = run
                    for j in range(NT):
                        pL = psL.tile([P, L], F32, name=f"pL{l}_{h}_{j}", tag="L")
                        qa_s = qaT[t][rb: rb + 9, j * P:(j + 1) * P]
                        for hf in range(2):
                            nc.tensor.matmul(
                                pL[:, hf * 512:(hf + 1) * 512], qa_s,
                                kaT[t][rb: rb + 9, hf * 512:(hf + 1) * 512],
                                tile_position=(rb, 0),
                            )
                        nc.scalar.activation(
                            out=e_h[:, j * L:(j + 1) * L], in_=pL, func=Act.Exp)
                        # bf16 TT-max runs at 2x on DVE; reduce_max only at 1x
                        if j == 1:
                            nc.vector.tensor_tensor(
                                out=run, in0=e_h[:, 0:L], in1=e_h[:, L:2 * L],
                                op=Alu.max)
                        elif j > 1:
                            nc.vector.tensor_tensor(
                                out=run, in0=run,
                                in1=e_h[:, j * L:(j + 1) * L], op=Alu.max)

                def maxred_head(h):
                    nc.vector.reduce_max(out=mx_s8[:, h: h + 1],
                                         in_=run_tiles[h],
                                         axis=mybir.AxisListType.X)

                def etv_head(h):
                    e_h = e_tiles[h]
                    T = X if h < 4 else Y
                    c = h % 4
                    for j in range(NT):
                        lhsT = vt32[:, j, h, :]
                        for hf in range(2):
                            nc.tensor.matmul(
                                T[32 * c: 32 * c + 32, hf * 512:(hf + 1) * 512],
                                lhsT,
                                e_h[:, j * L + hf * 512: j * L + (hf + 1) * 512],
                                start=(j == 0), stop=(j == NT - 1),
                                tile_position=(0, 32 * c),
                            )

                sXa = stats.tile([P, 1], F32, name=f"sXa{l}", tag="sXa")
                sYa = stats.tile([P, 1], F32, name=f"sYa{l}", tag="sYa")

                # PE order: logits(0), logits(1), etv(0), logits(2), etv(1), ...
                logits_head(0)
                for h in range(1, H):
                    logits_head(h)
                    maxred_head(h - 1)
                    if h == 6:
                        # X (heads 0-3) is complete; evacuate early
                        nc.vector.tensor_copy(attnTX, X)
                        nc.vector.reduce_sum(out=sXa, in_=X,
                                             axis=mybir.AxisListType.X)
                    etv_head(h - 1)
                maxred_head(7)
                etv_head(7)

                # ======== evacuate Y (unscaled) + colsum accum ========
                nc.scalar.copy(attnTY, Y)
                nc.vector.reduce_sum(out=sYa, in_=Y, axis=mybir.AxisListType.X)

                # ======== per-head stats -> G (AllReduce max) ========
                pS = psL.tile([P, L], F32, name=f"pS{l}", tag="L")
                nc.tensor.transpose(out=pS[0:H, 0:P], in_=mx_s8, identity=I128)
                statTm = stats.tile([H, P], F32, name=f"statTm{l}", tag="statTm")
                nc.vector.tensor_copy(statTm, pS[0:H, 0:P])
                mxh8 = stats.tile([H, 1], F32, name=f"mxh{l}", tag="mxh")
                nc.vector.reduce_max(out=mxh8, in_=statTm,
                                     axis=mybir.AxisListType.X)
                pS8 = psL.tile([P, L], F32, name=f"pS8{l}", tag="L")
                nc.tensor.matmul(pS8[0:H, 0:1], selX, sXa, start=True, stop=False)
                nc.tensor.matmul(pS8[0:H, 0:1], selY, sYa, start=False, stop=True)
                s8 = stats.tile([H, 1], F32, name=f"s8{l}", tag="s8")
                nc.vector.tensor_copy(s8, pS8[0:H, 0:1])
                rs8 = stats.tile([H, 1], F32, name=f"rs{l}", tag="rs")
                nc.vector.reciprocal(out=rs8, in_=s8)
                t8 = stats.tile([H, 1], F32, name=f"t8{l}", tag="t8")
                nc.vector.tensor_mul(t8, mxh8, rs8)
                nc.vector.tensor_mul(t8, t8, nz8)
                pT = psL.tile([P, L], F32, name=f"pT{l}", tag="L")
                nc.tensor.transpose(out=pT[0:1, 0:H], in_=t8,
                                    identity=I128[0:H, 0:H])
                t8row = stats.tile([1, H], F32, name=f"t8row{l}", tag="t8row")
                nc.vector.tensor_copy(t8row, pT[0:1, 0:H])
                gl = stats.tile([1, 1], F32, name=f"gl{l}", tag="gl")
                nc.vector.reduce_max(out=gl, in_=t8row, axis=mybir.AxisListType.X)

                # c8_loc = nz/s (local part; 1/G deferred to the z-add)
                c8 = stats.tile([H, 1], F32, name=f"c8{l}", tag="c8")
                nc.vector.tensor_mul(c8, rs8, nz8)
                pC = psL.tile([P, L], F32, name=f"pC{l}", tag="L")
                nc.tensor.matmul(pC[:, 0:1], e128X, c8, start=True, stop=True)
                nc.tensor.matmul(pC[:, 1:2], e128Y, c8, start=True, stop=True)
                c128 = stats.tile([P, 2], F32, name=f"c128{l}", tag="c128")
                nc.vector.tensor_copy(c128, pC[:, 0:2])
                woXs = stats.tile([P, D], BF16, name=f"woXs{l}", tag="woXs")
                nc.vector.tensor_scalar(out=woXs, in0=woX_t[l],
                                        scalar1=c128[:, 0:1], scalar2=None,
                                        op0=Alu.mult)
                woYs = stats.tile([P, D], BF16, name=f"woYs{l}", tag="woYs")
                nc.vector.tensor_scalar(out=woYs, in0=woY_t[l],
                                        scalar1=c128[:, 1:2], scalar2=None,
                                        op0=Alu.mult)

                cc_in = dramp.tile([1, 1], F32, name=f"cc_in{l}", tag=f"cc_in{l}")
                cc_out = dramp.tile([1, 1], F32, name=f"cc_out{l}",
                                    tag=f"cc_out{l}", addr_space="Shared")
                nc.gpsimd.dma_start(out=cc_in[:], in_=gl)
                nc.gpsimd.collective_compute(
                    "AllReduce", Alu.max,
                    replica_groups=[list(range(NCORES))],
                    ins=[cc_in.opt()], outs=[cc_out.opt()],
                )
                G = stats.tile([1, 1], F32, name=f"G{l}", tag=f"G{l}")
                nc.gpsimd.dma_start(out=G, in_=cc_out[:])

                # ==== Wo (runs during the AllReduce; output unscaled by G)
                pZ1 = psL.tile([P, L], F32, name=f"pZ1{l}", tag="L")
                for j in range(NT):
                    jsl = slice(j * D, (j + 1) * D)
                    csl = slice(j * P, (j + 1) * P)
                    nc.tensor.matmul(pZ1[:, jsl], attnTX[:, csl], woXs,
                                     start=True, stop=False)
                    nc.tensor.matmul(pZ1[:, jsl], attnTY[:, csl], woYs,
                                     start=False, stop=True)

                # rG = 1/G broadcast to all partitions
                pGb = psL.tile([P, L], F32, name=f"pGb{l}", tag="L")
                nc.tensor.matmul(pGb[:, 0:1], ones_t[0:1, :], G)
                rG = stats.tile([P, 1], F32, name=f"rG{l}", tag="rG")
                nc.vector.reciprocal(out=rG, in_=pGb[:, 0:1])

                # ==== LN helper (token-major, halves for PE overlap) ====
                def layernorm(pZ, res_all, gb, bb, oname, otag,
                              keepfold=False, zscale=None):
                    HJ = NT // 2
                    z = acts.tile([P, NT * D], F32, name=oname + "_z", tag="z")
                    sq = acts.tile([P, NT * D], F32, name=oname + "_sq", tag="sq")
                    sums = stats.tile([P, NT], F32, name=oname + "_su", tag="su")
                    sqs = stats.tile([P, NT], F32, name=oname + "_sq2", tag="sq2")
                    mu = stats.tile([P, NT], F32, name=oname + "_mu", tag="mu")
                    vv = stats.tile([P, NT], F32, name=oname + "_vv", tag="vv")
                    musq = stats.tile([P, NT], F32, name=oname + "_ms", tag="ms")
                    rstd = stats.tile([P, NT], F32, name=oname + "_rs", tag="rsd")
                    tq = stats.tile([P, NT], F32, name=oname + "_tq", tag="tq")
                    o = acts.tile([P, NT * D], F32, name=oname, tag=otag)
                    for hh in range(2):
                        csl = slice(hh * HJ * D, (hh + 1) * HJ * D)
                        jsl = slice(hh * HJ, (hh + 1) * HJ)
                        if zscale is not None:
                            nc.vector.scalar_tensor_tensor(
                                out=z[:, csl], in0=pZ[:, csl], scalar=zscale,
                                in1=res_all[:, csl], op0=Alu.mult, op1=Alu.add)
                        else:
                            nc.vector.tensor_add(z[:, csl], pZ[:, csl],
                                                 res_all[:, csl])
                        nc.vector.tensor_mul(sq[:, csl], z[:, csl], z[:, csl])
                        zj = z[:, csl].rearrange("p (j f) -> p j f", f=D)
                        nc.vector.reduce_sum(out=sums[:, jsl], in_=zj,
                                             axis=mybir.AxisListType.X)
                        nc.vector.reduce_sum(
                            out=sqs[:, jsl],
                            in_=sq[:, csl].rearrange("p (j f) -> p j f", f=D),
                            axis=mybir.AxisListType.X)
                        nc.vector.tensor_scalar(out=mu[:, jsl], in0=sums[:, jsl],
                                                scalar1=1.0 / D,
                                                scalar2=None, op0=Alu.mult)
                        nc.vector.tensor_scalar(out=vv[:, jsl], in0=sqs[:, jsl],
                                                scalar1=1.0 / D, scalar2=LN_EPS,
                                                op0=Alu.mult, op1=Alu.add)
                        nc.vector.tensor_mul(musq[:, jsl], mu[:, jsl], mu[:, jsl])
                        nc.vector.tensor_sub(vv[:, jsl], vv[:, jsl], musq[:, jsl])
                        # rsqrt: bit-trick seed + 3 Newton iterations
                        iv = vv[:, jsl].bitcast(mybir.dt.int32)
                        ir = rstd[:, jsl].bitcast(mybir.dt.int32)
                        nc.vector.tensor_scalar(out=ir, in0=iv, scalar1=1,
                                                scalar2=None,
                                                op0=Alu.logical_shift_right)
                        nc.vector.tensor_scalar(out=ir, in0=ir, scalar1=-1,
                                                scalar2=0x5F3759DF, op0=Alu.mult,
                                                op1=Alu.add)
                        for _ in range(3):
                            nc.vector.tensor_mul(tq[:, jsl], rstd[:, jsl],
                                                 rstd[:, jsl])
                            nc.vector.tensor_mul(tq[:, jsl], tq[:, jsl],
                                                 vv[:, jsl])
                            nc.vector.tensor_scalar(out=tq[:, jsl],
                                                    in0=tq[:, jsl], scalar1=-0.5,
                                                    scalar2=1.5, op0=Alu.mult,
                                                    op1=Alu.add)
                            nc.vector.tensor_mul(rstd[:, jsl], rstd[:, jsl],
                                                 tq[:, jsl])
                        oj = o[:, csl].rearrange("p (j f) -> p j f", f=D)
                        nc.vector.tensor_tensor(out=oj, in0=zj,
                                                in1=bc_f(mu[:, jsl], D),
                                                op=Alu.subtract)
                        nc.vector.tensor_tensor(out=oj, in0=oj,
                                                in1=bc_f(rstd[:, jsl], D),
                                                op=Alu.mult)
                        if keepfold:
                            nc.vector.tensor_mul(o[:, csl], o[:, csl],
                                                 gb[:, csl])
                            nc.vector.tensor_add(o[:, csl], o[:, csl],
                                                 bb[:, csl])
                        else:
                            nc.vector.tensor_tensor(out=oj, in0=oj,
                                                    in1=bc_j(gb, HJ),
                                                    op=Alu.mult)
                            nc.vector.tensor_tensor(out=oj, in0=oj,
                                                    in1=bc_j(bb, HJ),
                                                    op=Alu.add)
                    return o

                out1_all = layernorm(pZ1, x_bo, g1_t[l], be1_t[l],
                                     f"out1_{l}", "out1", zscale=rG)

                # ======== FFN ========
                pO = psL.tile([P, L], F32, name=f"pO{l}", tag="L")
                for hf in range(2):
                    for jj in range(NT // 2):
                        j = hf * (NT // 2) + jj
                        nc.tensor.transpose(
                            out=pO[0:D, j * P:(j + 1) * P],
                            in_=out1_all[:, j * D:(j + 1) * D], identity=I128,
                        )
                    csl = slice(hf * 512, (hf + 1) * 512)
                    nc.vector.tensor_copy(out1T[0:D, csl], pO[0:D, csl])

                h1 = []
                for i in range(2):
                    pH = psL.tile([P, L], F32, name=f"pH{l}_{i}", tag="L")
                    w1x = w1_t[l][:, i * P:(i + 1) * P]
                    for hf in range(2):
                        nc.tensor.matmul(pH[:, hf * 512:(hf + 1) * 512],
                                         w1x, out1T[:, hf * 512:(hf + 1) * 512])
                    h1x = acts.tile([P, L], BF16, name=f"h1_{l}_{i}",
                                    tag=f"h1_{i}")
                    for hf in range(2):
                        csl = slice(hf * 512, (hf + 1) * 512)
                        nc.vector.tensor_scalar(out=h1x[:, csl], in0=pH[:, csl],
                                                scalar1=0.0, scalar2=None,
                                                op0=Alu.max)
                    h1.append(h1x)

                pZ2 = psL.tile([P, L], F32, name=f"pZ2{l}", tag="L")
                for j in range(NT):
                    jsl = slice(j * D, (j + 1) * D)
                    csl = slice(j * P, (j + 1) * P)
                    nc.tensor.matmul(pZ2[:, jsl], ones_bf[0:1, csl], b2_t[l],
                                     start=True, stop=False)
                    nc.tensor.matmul(pZ2[:, jsl], h1[0][:, csl], w2_t[l][0],
                                     start=False, stop=False)
                    nc.tensor.matmul(pZ2[:, jsl], h1[1][:, csl], w2_t[l][1],
                                     start=False, stop=True)

                x_all = layernorm(pZ2, out1_all, g2k, be2k,
                                  f"x_next_{l}", "x", keepfold=True)

            dma(out=out.rearrange("(j p) f -> p j f", p=P),
                in_=x_all.rearrange("p (j f) -> p j f", f=D))

    return nc


_NC_CACHE = None


def _get_nc():
    global _NC_CACHE
    if _NC_CACHE is None:
        _NC_CACHE = build_bass()
    return _NC_CACHE


def _make_in_maps(inputs):
    x = np.asarray(inputs["x"], np.float32)
    protok = np.asarray(inputs["protok"])
    B = x.shape[0]
    keep = (protok != 0).astype(np.float32)

    Wq_in = np.asarray(inputs["Wq"], np.float32)
    Wk_in = np.asarray(inputs["Wk"], np.float32)
    bq = np.asarray(inputs["bq"], np.float32)
    bk = np.asarray(inputs["bk"], np.float32)

    Wq_aug = np.zeros((NL, NQT, XAUG, AUG), np.float32)
    Wk_aug = np.zeros((NL, NQT, XAUG, AUG), np.float32)
    for t in range(NQT):
        for m in range(HPT[t]):
            h = 4 * t + m
            cols = 32 * m + np.arange(DH)
            Wq_aug[:, t, 0:D, cols.min():cols.max() + 1] = \
                Wq_in[:, :, DH * h:DH * (h + 1)]
            Wk_aug[:, t, 0:D, cols.min():cols.max() + 1] = \
                Wk_in[:, :, DH * h:DH * (h + 1)]
            Wq_aug[:, t, D, cols] = bq[:, DH * h:DH * (h + 1)]
            Wk_aug[:, t, D, cols] = bk[:, DH * h:DH * (h + 1)]
            # ones row (q) / padneg rows (k) at 32m+8
            Wq_aug[:, t, D + 1, 32 * m + 8] = 1.0
            Wk_aug[:, t, D, 32 * m + 8] = NEG_BIG
            Wk_aug[:, t, D + 1, 32 * m + 8] = -NEG_BIG

    Wv_aug = np.zeros((NL, XAUG, D), np.float32)
    Wv_aug[:, 0:D, :] = np.asarray(inputs["Wv"], np.float32)
    Wv_aug[:, D, :] = np.asarray(inputs["bv"], np.float32)

    Wo = np.asarray(inputs["Wo"], np.float32)
    WoM = np.zeros((NL, 2, P, D), np.float32)
    for xy in range(2):
        for c in range(4):
            h = 4 * xy + c
            WoM[:, xy, 32 * c:32 * c + DH, :] = Wo[:, DH * h:DH * (h + 1), :]

    W1_aug = np.zeros((NL, XAUG - 1, DFF), np.float32)
    W1_aug[:, 0:D, :] = np.asarray(inputs["W1"], np.float32)
    W1_aug[:, D, :] = np.asarray(inputs["b1"], np.float32)
    W2_in = np.asarray(inputs["W2"], np.float32)
    W2p = np.stack([W2_in[:, 0:P, :], W2_in[:, P:DFF, :]], axis=1)

    e128 = np.zeros((2, DH, P), np.float32)
    sel128 = np.zeros((2, P, DH), np.float32)
    for xy in range(2):
        for c in range(4):
            h = 4 * xy + c
            e128[xy, h, 32 * c:32 * c + DH] = 1.0
            sel128[xy, 32 * c + DH, h] = 1.0

    shared = dict(
        Wq=Wq_aug.astype(BF),
        Wk=Wk_aug.astype(BF),
        Wv=Wv_aug.astype(BF),
        WoM=WoM,
        bov=np.ascontiguousarray(inputs["bo"], np.float32),
        W1=W1_aug.astype(BF),
        W2=W2p.astype(BF),
        b2_r=np.asarray(inputs["b2"], np.float32)[:, None, :].astype(BF),
        g1v=np.ascontiguousarray(inputs["g1"], np.float32),
        be1v=np.ascontiguousarray(inputs["be1"], np.float32),
        g2v=np.ascontiguousarray(inputs["g2"], np.float32),
        be2v=np.ascontiguousarray(inputs["be2"], np.float32),
        e128=e128,
        sel128=sel128,
    )
    in_maps = []
    for i in range(NCORES):
        b = i % B
        in_maps.append(dict(
            x=np.ascontiguousarray(x[b]),
            keepf=np.ascontiguousarray(keep[b]),
            keep_bf=np.ascontiguousarray(keep[b].astype(BF)),
            **shared,
        ))
    return in_maps


def run_on_hw(inputs, trace=False, **kwargs):
    from concourse.bass_utils import run_bass_kernel_spmd

    nc = _get_nc()
    if not nc.is_finalized():
        nc.finalize()
    in_maps = _make_in_maps(inputs)
    res = run_bass_kernel_spmd(nc, in_maps, list(range(NCORES)), trace=trace,
                               **kwargs)
    outs = np.stack([res.results[i]["out"] for i in range(NCORES)], axis=0)
    return outs.astype(np.float32), res


def kernel(**inputs):
    outs, _ = run_on_hw(inputs, trace=False)
    return outs
